# revision 42
# baseline (speedup 1.0000x reference)
"""Trainium2 Bass kernel for LoRA linear: y = x @ (W + 2*B@A).T + b.

Full inputs: x (8, 2048, 2048) f32, W (2048, 2048) f32, b (2048,) f32,
B (2048, 16) f32, A (16, 2048) f32.  Output (8, 2048, 2048) f32.

Sharding: data-parallel over the batch dim — core i computes
y[i] = x[i] @ w.T + b with the merged weight w = W + 2*B@A.

Per-core kernel (bf16 TensorEngine compute, f32 accumulate):
  The PE is power-throttled to 1.2GHz (HAM k=4) whenever several
  engines + DMA run hot together, and only sustains 2.4GHz on a
  near-pure matmul stream.  A front-loaded weight/transpose phase
  therefore runs at ~half speed no matter how it is scheduled.  So
  the GEMM is restructured OC-OUTER: pass oc sweeps all 16 row
  tiles for one 512-wide output bank and needs only W row-blocks
  4oc..4oc+3; each group's prep (rank-16 delta, DVE merge, 16
  transposes) is emitted sprinkled inside the PREVIOUS pass, keeping
  non-PE engine duty low everywhere.  All 16 transposed x tiles stay
  resident in SBUF (xT 64K/part + wT 64K/part).

  Transposes are regular matmuls against the identity (f32 PSUM,
  numerically exact for bf16 data): they pipeline at the matmul rate
  and avoid is_transpose<->matmul mode switches.

  phase 0 (gpsimd software-DGE queue, ~12us cold start): A, B staged
  bf16, x0, bias broadcast, x1.. cast-loads.  W row-blocks f32 on the
  two HWDGE queues (evens scalar, odds sync).  Per-bank y stores on
  the sync queue.
"""

import numpy as np

import concourse.bacc as bacc
import concourse.mybir as mybir
import concourse.tile as tile
from concourse import masks
from concourse.bass_utils import run_bass_kernel_spmd

N_CORES = 8
BATCH, S, D = 8, 2048, 2048
RANK = 16
SCALE = 2.0  # alpha / rank = 32 / 16
P = 128  # partitions
FREE = 512  # f32 elems per PSUM bank
ND = D // P  # 16 contraction tiles
NS = S // P  # 16 row tiles per core
NO = D // FREE  # 4 output banks per row tile
NG = ND // 4  # 4 groups of 4

F32 = mybir.dt.float32
BF16 = mybir.dt.bfloat16


def build_nc():
    nc = bacc.Bacc(
        "TRN2", target_bir_lowering=False, debug=False, num_devices=N_CORES
    )
    x_d = nc.dram_tensor("x", [S, D], F32, kind="ExternalInput").ap()
    W_d = nc.dram_tensor("W", [D, D], F32, kind="ExternalInput").ap()
    b_d = nc.dram_tensor("b", [D], F32, kind="ExternalInput").ap()
    B_d = nc.dram_tensor("B", [D, RANK], F32, kind="ExternalInput").ap()
    A_d = nc.dram_tensor("A", [RANK, D], F32, kind="ExternalInput").ap()
    out_d = nc.dram_tensor("out", [S, D], F32, kind="ExternalOutput").ap()

    with tile.TileContext(nc) as tc:
        with (
            tc.tile_pool(name="singles", bufs=1) as singles,
            tc.tile_pool(name="wt", bufs=1) as wtp,
            tc.tile_pool(name="xt", bufs=1) as xtp,
        ):
            ident = singles.tile([P, P], BF16)
            masks.make_identity(nc, ident[:])

            A_sb = singles.tile([RANK, D], BF16)
            nc.gpsimd.dma_start(out=A_sb[:], in_=A_d[:])

            B2T = singles.tile([RANK, D], BF16)
            Bs = singles.tile([P, ND * RANK], BF16)
            nc.gpsimd.dma_start(
                out=Bs[:], in_=B_d.rearrange("(t p) r -> p t r", p=P)
            )

            bb = singles.tile([P, D], BF16)

            # both transposed operands fully resident in SBUF
            wT = wtp.tile([P, ND, D], BF16)  # wT[q, dt, o] = w[o, dt*128+q]
            xT = xtp.tile([P, ND, S], BF16)  # xT[q, dt, s] = x[s, dt*128+q]

            with (
                tc.tile_pool(name="wrow", bufs=4) as wrowp,
                tc.tile_pool(name="w16", bufs=2) as w16p,
                tc.tile_pool(name="xstage", bufs=5) as xstage,
                tc.tile_pool(name="yout", bufs=3) as youtp,
                tc.tile_pool(name="dpsum", bufs=3, space="PSUM") as dpsum,
                tc.tile_pool(name="tpsum", bufs=3, space="PSUM") as tpsum,
                tc.tile_pool(name="gpsum", bufs=2, space="PSUM") as gpsum,
            ):
                def x_load(st):
                    xs = xstage.tile([P, D], BF16, tag="xs")
                    nc.gpsimd.dma_start(
                        out=xs[:], in_=x_d[st * P : (st + 1) * P, :]
                    )
                    return xs

                # gpsimd queue order: A, B, x0, bias, x1, x2, ...
                xs_tiles = {0: x_load(0)}
                nc.gpsimd.dma_start(
                    out=bb[:], in_=b_d[None, :].broadcast_to([P, D])
                )
                xs_tiles[1] = x_load(1)
                xs_tiles[2] = x_load(2)

                # W row-blocks, evens on scalar / odds on sync; pool-gated
                # so later groups stream in during the GEMM passes
                wrows = []
                for ot in range(ND):
                    wrow = wrowp.tile([P, D], F32, tag="wrow")
                    eng = nc.scalar if ot % 2 == 0 else nc.sync
                    eng.dma_start(
                        out=wrow[:], in_=W_d[ot * P : (ot + 1) * P, :]
                    )
                    wrows.append(wrow)

                # 2*B.T: transpose-by-matmul of the staged B tiles
                for g in range(NG):
                    bps = tpsum.tile([RANK, 4 * P], F32, tag="tp")
                    for j in range(4):
                        t = 4 * g + j
                        nc.tensor.matmul(
                            bps[:, j * P : (j + 1) * P],
                            Bs[:, t * RANK : (t + 1) * RANK],
                            ident[:],
                            start=True,
                            stop=True,
                        )
                    nc.vector.tensor_scalar_mul(
                        B2T[:, g * 4 * P : (g + 1) * 4 * P], bps[:], SCALE
                    )

                def x_transpose(st):
                    xs = xs_tiles.pop(st) if st in xs_tiles else x_load(st)
                    for g in range(NG):
                        tp = tpsum.tile([P, 4 * P], F32, tag="tp")
                        for j in range(4):
                            dt = 4 * g + j
                            nc.tensor.matmul(
                                tp[:, j * P : (j + 1) * P],
                                xs[:, dt * P : (dt + 1) * P],
                                ident[:],
                                start=True,
                                stop=True,
                            )
                        nc.scalar.copy(
                            xT[:, 4 * g : 4 * (g + 1), st * P : (st + 1) * P],
                            tp[:],
                        )

                def w_compute(ot):
                    w16 = w16p.tile([P, D], BF16, tag="w16")
                    dps = [
                        dpsum.tile([P, FREE], F32, tag="dp", name=f"dp{ot}_{g}")
                        for g in range(NG)
                    ]
                    for g in range(NG):
                        nc.tensor.matmul(
                            dps[g][:],
                            B2T[:, ot * P : (ot + 1) * P],
                            A_sb[:, g * FREE : (g + 1) * FREE],
                            start=True,
                            stop=True,
                        )
                    for g in range(NG):
                        nc.vector.tensor_add(
                            w16[:, g * FREE : (g + 1) * FREE],
                            dps[g][:],
                            wrows[ot][:, g * FREE : (g + 1) * FREE],
                        )
                    for g in range(NG):
                        tp = tpsum.tile([P, 4 * P], F32, tag="tp")
                        for j in range(4):
                            dt = 4 * g + j
                            nc.tensor.matmul(
                                tp[:, j * P : (j + 1) * P],
                                w16[:, dt * P : (dt + 1) * P],
                                ident[:],
                                start=True,
                                stop=True,
                            )
                        nc.vector.tensor_scalar_mul(
                            wT[:, 4 * g : 4 * (g + 1), ot * P : (ot + 1) * P],
                            tp[:],
                            1.0,
                        )

                # group 0 prep + the first x tiles up front
                for ot in range(4):
                    w_compute(ot)
                    x_transpose(ot)

                # ---- oc-outer GEMM: pass oc needs W blocks 4oc..4oc+3;
                # group oc+1 prep and remaining x transposes are sprinkled
                # inside pass oc ----
                nxt_x = 4
                nxt_w = 4
                for oc in range(NO):
                    for st in range(NS):
                        if oc == 0 and st >= 2 and nxt_x < NS:
                            x_transpose(nxt_x)
                            nxt_x += 1
                        wsts = (8, 10, 12, 14) if oc == 0 else (4, 7, 10, 13)
                        if st in wsts and nxt_w < ND:
                            w_compute(nxt_w)
                            nxt_w += 1
                        gp = gpsum.tile([P, FREE], F32)
                        for dt in range(ND):
                            nc.tensor.matmul(
                                gp[:],
                                xT[:, dt, st * P : (st + 1) * P],
                                wT[:, dt, oc * FREE : (oc + 1) * FREE],
                                start=(dt == 0),
                                stop=(dt == ND - 1),
                            )
                        ys = youtp.tile([P, FREE], F32, tag="ys")
                        nc.vector.tensor_add(
                            ys[:], gp[:], bb[:, oc * FREE : (oc + 1) * FREE]
                        )
                        nc.sync.dma_start(
                            out=out_d[
                                st * P : (st + 1) * P,
                                oc * FREE : (oc + 1) * FREE,
                            ],
                            in_=ys[:],
                        )

    nc.compile()
    return nc


_NC_CACHE = None


def _get_nc():
    global _NC_CACHE
    if _NC_CACHE is None:
        _NC_CACHE = build_nc()
    return _NC_CACHE


def make_in_maps(x, W, b, B, A):
    x = np.ascontiguousarray(x, dtype=np.float32)
    W = np.ascontiguousarray(W, dtype=np.float32)
    b = np.ascontiguousarray(b, dtype=np.float32)
    B = np.ascontiguousarray(B, dtype=np.float32)
    A = np.ascontiguousarray(A, dtype=np.float32)
    return [
        {"x": x[i], "W": W, "b": b, "B": B, "A": A} for i in range(N_CORES)
    ]


def run(inputs, **spmd_kwargs):
    """Run the SPMD kernel; returns (output, BassKernelResults)."""
    nc = _get_nc()
    in_maps = make_in_maps(**inputs)
    res = run_bass_kernel_spmd(nc, in_maps, core_ids=list(range(N_CORES)), **spmd_kwargs)
    out = np.stack([res.results[i]["out"] for i in range(N_CORES)]).astype(np.float32)
    return out, res


def kernel(x, W, b, B, A):
    out, _ = run({"x": x, "W": W, "b": b, "B": B, "A": A})
    return out


# revision 43
# speedup vs baseline: 1.0094x; 1.0094x over previous
"""Trainium2 Bass kernel for LoRA linear: y = x @ (W + 2*B@A).T + b.

Full inputs: x (8, 2048, 2048) f32, W (2048, 2048) f32, b (2048,) f32,
B (2048, 16) f32, A (16, 2048) f32.  Output (8, 2048, 2048) f32.

Sharding: data-parallel over the batch dim — core i computes
y[i] = x[i] @ w.T + b with the merged weight w = W + 2*B@A.

Per-core kernel (bf16 TensorEngine compute, f32 accumulate):
  The PE is power-throttled to 1.2GHz (HAM k=4) whenever several
  engines + DMA run hot together, and only sustains 2.4GHz on a
  near-pure matmul stream.  A front-loaded weight/transpose phase
  therefore runs at ~half speed no matter how it is scheduled.  So
  the GEMM is restructured OC-OUTER: pass oc sweeps all 16 row
  tiles for one 512-wide output bank and needs only W row-blocks
  4oc..4oc+3; each group's prep (rank-16 delta, DVE merge, 16
  transposes) is emitted sprinkled inside the PREVIOUS pass, keeping
  non-PE engine duty low everywhere.  All 16 transposed x tiles stay
  resident in SBUF (xT 64K/part + wT 64K/part).

  Transposes are regular matmuls against the identity (f32 PSUM,
  numerically exact for bf16 data): they pipeline at the matmul rate
  and avoid is_transpose<->matmul mode switches.

  phase 0 (gpsimd software-DGE queue, ~12us cold start): A, B staged
  bf16, x0, bias broadcast, x1.. cast-loads.  W row-blocks f32 on the
  two HWDGE queues (evens scalar, odds sync).  Per-bank y stores on
  the sync queue.
"""

import numpy as np

import concourse.bacc as bacc
import concourse.mybir as mybir
import concourse.tile as tile
from concourse import masks
from concourse.bass_utils import run_bass_kernel_spmd

N_CORES = 8
BATCH, S, D = 8, 2048, 2048
RANK = 16
SCALE = 2.0  # alpha / rank = 32 / 16
P = 128  # partitions
FREE = 512  # f32 elems per PSUM bank
ND = D // P  # 16 contraction tiles
NS = S // P  # 16 row tiles per core
NO = D // FREE  # 4 output banks per row tile
NG = ND // 4  # 4 groups of 4

F32 = mybir.dt.float32
BF16 = mybir.dt.bfloat16


def build_nc():
    nc = bacc.Bacc(
        "TRN2", target_bir_lowering=False, debug=False, num_devices=N_CORES
    )
    x_d = nc.dram_tensor("x", [S, D], F32, kind="ExternalInput").ap()
    W_d = nc.dram_tensor("W", [D, D], F32, kind="ExternalInput").ap()
    b_d = nc.dram_tensor("b", [D], F32, kind="ExternalInput").ap()
    B_d = nc.dram_tensor("B", [D, RANK], F32, kind="ExternalInput").ap()
    A_d = nc.dram_tensor("A", [RANK, D], F32, kind="ExternalInput").ap()
    out_d = nc.dram_tensor("out", [S, D], F32, kind="ExternalOutput").ap()

    with tile.TileContext(nc) as tc:
        with (
            tc.tile_pool(name="singles", bufs=1) as singles,
            tc.tile_pool(name="wt", bufs=1) as wtp,
            tc.tile_pool(name="xt", bufs=1) as xtp,
        ):
            ident = singles.tile([P, P], BF16)
            masks.make_identity(nc, ident[:])

            A_sb = singles.tile([RANK, D], BF16)
            nc.gpsimd.dma_start(out=A_sb[:], in_=A_d[:])

            B2T = singles.tile([RANK, D], BF16)
            Bs = singles.tile([P, ND * RANK], BF16)
            nc.gpsimd.dma_start(
                out=Bs[:], in_=B_d.rearrange("(t p) r -> p t r", p=P)
            )

            bb = singles.tile([P, D], BF16)

            # both transposed operands fully resident in SBUF
            wT = wtp.tile([P, ND, D], BF16)  # wT[q, dt, o] = w[o, dt*128+q]
            xT = xtp.tile([P, ND, S], BF16)  # xT[q, dt, s] = x[s, dt*128+q]

            with (
                tc.tile_pool(name="wrow", bufs=3) as wrowp,
                tc.tile_pool(name="w16", bufs=2) as w16p,
                tc.tile_pool(name="xstage", bufs=6) as xstage,
                tc.tile_pool(name="yout", bufs=4) as youtp,
                tc.tile_pool(name="dpsum", bufs=3, space="PSUM") as dpsum,
                tc.tile_pool(name="tpsum", bufs=3, space="PSUM") as tpsum,
                tc.tile_pool(name="gpsum", bufs=2, space="PSUM") as gpsum,
            ):
                def x_load(st):
                    xs = xstage.tile([P, D], BF16, tag="xs")
                    nc.gpsimd.dma_start(
                        out=xs[:], in_=x_d[st * P : (st + 1) * P, :]
                    )
                    return xs

                # gpsimd queue order: A, B, x0, bias, x1, x2, ...
                xs_tiles = {0: x_load(0)}
                nc.gpsimd.dma_start(
                    out=bb[:], in_=b_d[None, :].broadcast_to([P, D])
                )
                xs_tiles[1] = x_load(1)
                xs_tiles[2] = x_load(2)

                # W row-blocks, evens on scalar / odds on sync; pool-gated
                # so later groups stream in during the GEMM passes
                wrows = []
                for ot in range(ND):
                    wrow = wrowp.tile([P, D], F32, tag="wrow")
                    eng = nc.scalar if ot % 2 == 0 else nc.sync
                    eng.dma_start(
                        out=wrow[:], in_=W_d[ot * P : (ot + 1) * P, :]
                    )
                    wrows.append(wrow)

                # 2*B.T: transpose-by-matmul of the staged B tiles
                for g in range(NG):
                    bps = tpsum.tile([RANK, 4 * P], F32, tag="tp")
                    for j in range(4):
                        t = 4 * g + j
                        nc.tensor.matmul(
                            bps[:, j * P : (j + 1) * P],
                            Bs[:, t * RANK : (t + 1) * RANK],
                            ident[:],
                            start=True,
                            stop=True,
                        )
                    nc.vector.tensor_scalar_mul(
                        B2T[:, g * 4 * P : (g + 1) * 4 * P], bps[:], SCALE
                    )

                def x_transpose(st):
                    xs = xs_tiles.pop(st) if st in xs_tiles else x_load(st)
                    for g in range(NG):
                        tp = tpsum.tile([P, 4 * P], F32, tag="tp")
                        for j in range(4):
                            dt = 4 * g + j
                            nc.tensor.matmul(
                                tp[:, j * P : (j + 1) * P],
                                xs[:, dt * P : (dt + 1) * P],
                                ident[:],
                                start=True,
                                stop=True,
                            )
                        nc.scalar.copy(
                            xT[:, 4 * g : 4 * (g + 1), st * P : (st + 1) * P],
                            tp[:],
                        )

                def w_compute(ot):
                    w16 = w16p.tile([P, D], BF16, tag="w16")
                    dps = [
                        dpsum.tile([P, FREE], F32, tag="dp", name=f"dp{ot}_{g}")
                        for g in range(NG)
                    ]
                    for g in range(NG):
                        nc.tensor.matmul(
                            dps[g][:],
                            B2T[:, ot * P : (ot + 1) * P],
                            A_sb[:, g * FREE : (g + 1) * FREE],
                            start=True,
                            stop=True,
                        )
                    for g in range(NG):
                        nc.vector.tensor_add(
                            w16[:, g * FREE : (g + 1) * FREE],
                            dps[g][:],
                            wrows[ot][:, g * FREE : (g + 1) * FREE],
                        )
                    for g in range(NG):
                        tp = tpsum.tile([P, 4 * P], F32, tag="tp")
                        for j in range(4):
                            dt = 4 * g + j
                            nc.tensor.matmul(
                                tp[:, j * P : (j + 1) * P],
                                w16[:, dt * P : (dt + 1) * P],
                                ident[:],
                                start=True,
                                stop=True,
                            )
                        nc.vector.tensor_scalar_mul(
                            wT[:, 4 * g : 4 * (g + 1), ot * P : (ot + 1) * P],
                            tp[:],
                            1.0,
                        )

                # group 0 prep + the first x tiles up front
                for ot in range(4):
                    w_compute(ot)
                    x_transpose(ot)

                # ---- oc-outer GEMM: pass oc needs W blocks 4oc..4oc+3;
                # group oc+1 prep and remaining x transposes are sprinkled
                # inside pass oc ----
                nxt_x = 4
                nxt_w = 4
                for oc in range(NO):
                    for st in range(NS):
                        if oc == 0 and st >= 2 and nxt_x < NS:
                            x_transpose(nxt_x)
                            nxt_x += 1
                        wsts = (8, 10, 12, 14) if oc == 0 else (4, 7, 10, 13)
                        if st in wsts and nxt_w < ND:
                            w_compute(nxt_w)
                            nxt_w += 1
                        gp = gpsum.tile([P, FREE], F32)
                        for dt in range(ND):
                            nc.tensor.matmul(
                                gp[:],
                                xT[:, dt, st * P : (st + 1) * P],
                                wT[:, dt, oc * FREE : (oc + 1) * FREE],
                                start=(dt == 0),
                                stop=(dt == ND - 1),
                            )
                        ys = youtp.tile([P, FREE], F32, tag="ys")
                        nc.vector.tensor_add(
                            ys[:], gp[:], bb[:, oc * FREE : (oc + 1) * FREE]
                        )
                        nc.sync.dma_start(
                            out=out_d[
                                st * P : (st + 1) * P,
                                oc * FREE : (oc + 1) * FREE,
                            ],
                            in_=ys[:],
                        )

    nc.compile()
    return nc


_NC_CACHE = None


def _get_nc():
    global _NC_CACHE
    if _NC_CACHE is None:
        _NC_CACHE = build_nc()
    return _NC_CACHE


def make_in_maps(x, W, b, B, A):
    x = np.ascontiguousarray(x, dtype=np.float32)
    W = np.ascontiguousarray(W, dtype=np.float32)
    b = np.ascontiguousarray(b, dtype=np.float32)
    B = np.ascontiguousarray(B, dtype=np.float32)
    A = np.ascontiguousarray(A, dtype=np.float32)
    return [
        {"x": x[i], "W": W, "b": b, "B": B, "A": A} for i in range(N_CORES)
    ]


def run(inputs, **spmd_kwargs):
    """Run the SPMD kernel; returns (output, BassKernelResults)."""
    nc = _get_nc()
    in_maps = make_in_maps(**inputs)
    res = run_bass_kernel_spmd(nc, in_maps, core_ids=list(range(N_CORES)), **spmd_kwargs)
    out = np.stack([res.results[i]["out"] for i in range(N_CORES)]).astype(np.float32)
    return out, res


def kernel(x, W, b, B, A):
    out, _ = run({"x": x, "W": W, "b": b, "B": B, "A": A})
    return out


# revision 45
# speedup vs baseline: 1.0133x; 1.0038x over previous
"""Trainium2 Bass kernel for LoRA linear: y = x @ (W + 2*B@A).T + b.

Full inputs: x (8, 2048, 2048) f32, W (2048, 2048) f32, b (2048,) f32,
B (2048, 16) f32, A (16, 2048) f32.  Output (8, 2048, 2048) f32.

Sharding: data-parallel over the batch dim — core i computes
y[i] = x[i] @ w.T + b with the merged weight w = W + 2*B@A.

Per-core kernel (bf16 TensorEngine compute, f32 accumulate):
  The PE is power-throttled to 1.2GHz (HAM k=4) whenever several
  engines + DMA run hot together, and only sustains 2.4GHz on a
  near-pure matmul stream.  A front-loaded weight/transpose phase
  therefore runs at ~half speed no matter how it is scheduled.  So
  the GEMM is restructured OC-OUTER: pass oc sweeps all 16 row
  tiles for one 512-wide output bank and needs only W row-blocks
  4oc..4oc+3; each group's prep (rank-16 delta, DVE merge, 16
  transposes) is emitted sprinkled inside the PREVIOUS pass, keeping
  non-PE engine duty low everywhere.  All 16 transposed x tiles stay
  resident in SBUF (xT 64K/part + wT 64K/part).

  Transposes are regular matmuls against the identity (f32 PSUM,
  numerically exact for bf16 data): they pipeline at the matmul rate
  and avoid is_transpose<->matmul mode switches.

  phase 0 (gpsimd software-DGE queue, ~12us cold start): A, B staged
  bf16, x0, bias broadcast, x1.. cast-loads.  W row-blocks f32 on the
  two HWDGE queues (evens scalar, odds sync).  Per-bank y stores on
  the sync queue.
"""

import numpy as np

import concourse.bacc as bacc
import concourse.mybir as mybir
import concourse.tile as tile
from concourse import masks
from concourse.bass_utils import run_bass_kernel_spmd

N_CORES = 8
BATCH, S, D = 8, 2048, 2048
RANK = 16
SCALE = 2.0  # alpha / rank = 32 / 16
P = 128  # partitions
FREE = 512  # f32 elems per PSUM bank
ND = D // P  # 16 contraction tiles
NS = S // P  # 16 row tiles per core
NO = D // FREE  # 4 output banks per row tile
NG = ND // 4  # 4 groups of 4

F32 = mybir.dt.float32
BF16 = mybir.dt.bfloat16


def build_nc():
    nc = bacc.Bacc(
        "TRN2", target_bir_lowering=False, debug=False, num_devices=N_CORES
    )
    x_d = nc.dram_tensor("x", [S, D], F32, kind="ExternalInput").ap()
    W_d = nc.dram_tensor("W", [D, D], F32, kind="ExternalInput").ap()
    b_d = nc.dram_tensor("b", [D], F32, kind="ExternalInput").ap()
    BT_d = nc.dram_tensor("BT", [RANK, D], F32, kind="ExternalInput").ap()
    A_d = nc.dram_tensor("A", [RANK, D], F32, kind="ExternalInput").ap()
    out_d = nc.dram_tensor("out", [S, D], F32, kind="ExternalOutput").ap()

    with tile.TileContext(nc) as tc:
        with (
            tc.tile_pool(name="singles", bufs=1) as singles,
            tc.tile_pool(name="wt", bufs=1) as wtp,
            tc.tile_pool(name="xt", bufs=1) as xtp,
        ):
            ident = singles.tile([P, P], BF16)
            masks.make_identity(nc, ident[:])

            A_sb = singles.tile([RANK, D], BF16)
            nc.gpsimd.dma_start(out=A_sb[:], in_=A_d[:])

            # B.T is repacked on the host (layout only): a contiguous
            # [16, 2048] cast-load replaces the 2048x64B descriptor bomb
            # and the on-chip B2T transpose build
            B2T = singles.tile([RANK, D], BF16)
            BTs = singles.tile([RANK, D], BF16)
            nc.gpsimd.dma_start(out=BTs[:], in_=BT_d[:])

            bb = singles.tile([P, D], BF16)

            # both transposed operands fully resident in SBUF
            wT = wtp.tile([P, ND, D], BF16)  # wT[q, dt, o] = w[o, dt*128+q]
            xT = xtp.tile([P, ND, S], BF16)  # xT[q, dt, s] = x[s, dt*128+q]

            with (
                tc.tile_pool(name="wrow", bufs=3) as wrowp,
                tc.tile_pool(name="w16", bufs=2) as w16p,
                tc.tile_pool(name="xstage", bufs=6) as xstage,
                tc.tile_pool(name="yout", bufs=3) as youtp,
                tc.tile_pool(name="dpsum", bufs=3, space="PSUM") as dpsum,
                tc.tile_pool(name="tpsum", bufs=3, space="PSUM") as tpsum,
                tc.tile_pool(name="gpsum", bufs=2, space="PSUM") as gpsum,
            ):
                def x_load(st):
                    xs = xstage.tile([P, D], BF16, tag="xs")
                    nc.gpsimd.dma_start(
                        out=xs[:], in_=x_d[st * P : (st + 1) * P, :]
                    )
                    return xs

                # gpsimd queue order: A, B, x0, bias, x1, x2, ...
                xs_tiles = {0: x_load(0)}
                nc.gpsimd.dma_start(
                    out=bb[:], in_=b_d[None, :].broadcast_to([P, D])
                )
                xs_tiles[1] = x_load(1)
                xs_tiles[2] = x_load(2)

                # W row-blocks, evens on scalar / odds on sync; pool-gated
                # so later groups stream in during the GEMM passes
                wrows = []
                for ot in range(ND):
                    wrow = wrowp.tile([P, D], F32, tag="wrow")
                    eng = nc.scalar if ot % 2 == 0 else nc.sync
                    eng.dma_start(
                        out=wrow[:], in_=W_d[ot * P : (ot + 1) * P, :]
                    )
                    wrows.append(wrow)

                # 2*B.T via one DVE scale of the host-repacked B.T
                nc.vector.tensor_scalar_mul(B2T[:], BTs[:], SCALE)

                def x_transpose(st):
                    xs = xs_tiles.pop(st) if st in xs_tiles else x_load(st)
                    for g in range(NG):
                        tp = tpsum.tile([P, 4 * P], F32, tag="tp")
                        for j in range(4):
                            dt = 4 * g + j
                            nc.tensor.matmul(
                                tp[:, j * P : (j + 1) * P],
                                xs[:, dt * P : (dt + 1) * P],
                                ident[:],
                                start=True,
                                stop=True,
                            )
                        nc.scalar.copy(
                            xT[:, 4 * g : 4 * (g + 1), st * P : (st + 1) * P],
                            tp[:],
                        )

                def w_compute(ot):
                    w16 = w16p.tile([P, D], BF16, tag="w16")
                    dps = [
                        dpsum.tile([P, FREE], F32, tag="dp", name=f"dp{ot}_{g}")
                        for g in range(NG)
                    ]
                    for g in range(NG):
                        nc.tensor.matmul(
                            dps[g][:],
                            B2T[:, ot * P : (ot + 1) * P],
                            A_sb[:, g * FREE : (g + 1) * FREE],
                            start=True,
                            stop=True,
                        )
                    for g in range(NG):
                        nc.vector.tensor_add(
                            w16[:, g * FREE : (g + 1) * FREE],
                            dps[g][:],
                            wrows[ot][:, g * FREE : (g + 1) * FREE],
                        )
                    for g in range(NG):
                        tp = tpsum.tile([P, 4 * P], F32, tag="tp")
                        for j in range(4):
                            dt = 4 * g + j
                            nc.tensor.matmul(
                                tp[:, j * P : (j + 1) * P],
                                w16[:, dt * P : (dt + 1) * P],
                                ident[:],
                                start=True,
                                stop=True,
                            )
                        nc.vector.tensor_scalar_mul(
                            wT[:, 4 * g : 4 * (g + 1), ot * P : (ot + 1) * P],
                            tp[:],
                            1.0,
                        )

                # group 0 prep + the first x tiles up front
                for ot in range(4):
                    w_compute(ot)
                    x_transpose(ot)

                # ---- oc-outer GEMM: pass oc needs W blocks 4oc..4oc+3;
                # group oc+1 prep and remaining x transposes are sprinkled
                # inside pass oc ----
                nxt_x = 4
                nxt_w = 4
                for oc in range(NO):
                    for st in range(NS):
                        if oc == 0 and st >= 2 and nxt_x < NS:
                            x_transpose(nxt_x)
                            nxt_x += 1
                        wsts = (8, 10, 12, 14) if oc == 0 else (4, 7, 10, 13)
                        if st in wsts and nxt_w < ND:
                            w_compute(nxt_w)
                            nxt_w += 1
                        gp = gpsum.tile([P, FREE], F32)
                        for dt in range(ND):
                            nc.tensor.matmul(
                                gp[:],
                                xT[:, dt, st * P : (st + 1) * P],
                                wT[:, dt, oc * FREE : (oc + 1) * FREE],
                                start=(dt == 0),
                                stop=(dt == ND - 1),
                            )
                        ys = youtp.tile([P, FREE], F32, tag="ys")
                        nc.vector.tensor_add(
                            ys[:], gp[:], bb[:, oc * FREE : (oc + 1) * FREE]
                        )
                        nc.sync.dma_start(
                            out=out_d[
                                st * P : (st + 1) * P,
                                oc * FREE : (oc + 1) * FREE,
                            ],
                            in_=ys[:],
                        )

    nc.compile()
    return nc


_NC_CACHE = None


def _get_nc():
    global _NC_CACHE
    if _NC_CACHE is None:
        _NC_CACHE = build_nc()
    return _NC_CACHE


def make_in_maps(x, W, b, B, A):
    x = np.ascontiguousarray(x, dtype=np.float32)
    W = np.ascontiguousarray(W, dtype=np.float32)
    b = np.ascontiguousarray(b, dtype=np.float32)
    B = np.ascontiguousarray(B, dtype=np.float32)
    A = np.ascontiguousarray(A, dtype=np.float32)
    BT = np.ascontiguousarray(B.T)  # layout repack only
    return [
        {"x": x[i], "W": W, "b": b, "BT": BT, "A": A} for i in range(N_CORES)
    ]


def run(inputs, **spmd_kwargs):
    """Run the SPMD kernel; returns (output, BassKernelResults)."""
    nc = _get_nc()
    in_maps = make_in_maps(**inputs)
    res = run_bass_kernel_spmd(nc, in_maps, core_ids=list(range(N_CORES)), **spmd_kwargs)
    out = np.stack([res.results[i]["out"] for i in range(N_CORES)]).astype(np.float32)
    return out, res


def kernel(x, W, b, B, A):
    out, _ = run({"x": x, "W": W, "b": b, "B": B, "A": A})
    return out


# revision 46
# speedup vs baseline: 1.0214x; 1.0080x over previous
"""Trainium2 Bass kernel for LoRA linear: y = x @ (W + 2*B@A).T + b.

Full inputs: x (8, 2048, 2048) f32, W (2048, 2048) f32, b (2048,) f32,
B (2048, 16) f32, A (16, 2048) f32.  Output (8, 2048, 2048) f32.

Sharding: data-parallel over the batch dim — core i computes
y[i] = x[i] @ w.T + b with the merged weight w = W + 2*B@A.

Per-core kernel (bf16 TensorEngine compute, f32 accumulate):
  The PE is power-throttled to 1.2GHz (HAM k=4) whenever several
  engines + DMA run hot together, and only sustains 2.4GHz on a
  near-pure matmul stream.  A front-loaded weight/transpose phase
  therefore runs at ~half speed no matter how it is scheduled.  So
  the GEMM is restructured OC-OUTER: pass oc sweeps all 16 row
  tiles for one 512-wide output bank and needs only W row-blocks
  4oc..4oc+3; each group's prep (rank-16 delta, DVE merge, 16
  transposes) is emitted sprinkled inside the PREVIOUS pass, keeping
  non-PE engine duty low everywhere.  All 16 transposed x tiles stay
  resident in SBUF (xT 64K/part + wT 64K/part).

  Transposes are regular matmuls against the identity (f32 PSUM,
  numerically exact for bf16 data): they pipeline at the matmul rate
  and avoid is_transpose<->matmul mode switches.

  phase 0 (gpsimd software-DGE queue, ~12us cold start): A, B staged
  bf16, x0, bias broadcast, x1.. cast-loads.  W row-blocks f32 on the
  two HWDGE queues (evens scalar, odds sync).  Per-bank y stores on
  the sync queue.
"""

import numpy as np

import concourse.bacc as bacc
import concourse.mybir as mybir
import concourse.tile as tile
from concourse import masks
from concourse.bass_utils import run_bass_kernel_spmd

N_CORES = 8
BATCH, S, D = 8, 2048, 2048
RANK = 16
SCALE = 2.0  # alpha / rank = 32 / 16
P = 128  # partitions
FREE = 512  # f32 elems per PSUM bank
ND = D // P  # 16 contraction tiles
NS = S // P  # 16 row tiles per core
NO = D // FREE  # 4 output banks per row tile
NG = ND // 4  # 4 groups of 4

F32 = mybir.dt.float32
BF16 = mybir.dt.bfloat16


def build_nc():
    nc = bacc.Bacc(
        "TRN2", target_bir_lowering=False, debug=False, num_devices=N_CORES
    )
    x_d = nc.dram_tensor("x", [S, D], F32, kind="ExternalInput").ap()
    W_d = nc.dram_tensor("W", [D, D], F32, kind="ExternalInput").ap()
    b_d = nc.dram_tensor("b", [D], F32, kind="ExternalInput").ap()
    BT_d = nc.dram_tensor("BT", [RANK, D], F32, kind="ExternalInput").ap()
    A_d = nc.dram_tensor("A", [RANK, D], F32, kind="ExternalInput").ap()
    out_d = nc.dram_tensor("out", [S, D], F32, kind="ExternalOutput").ap()

    with tile.TileContext(nc) as tc:
        with (
            tc.tile_pool(name="singles", bufs=1) as singles,
            tc.tile_pool(name="wt", bufs=1) as wtp,
            tc.tile_pool(name="xt", bufs=1) as xtp,
        ):
            ident = singles.tile([P, P], BF16)
            masks.make_identity(nc, ident[:])

            A_sb = singles.tile([RANK, D], BF16)
            nc.gpsimd.dma_start(out=A_sb[:], in_=A_d[:])

            # B.T is repacked on the host (layout only): a contiguous
            # [16, 2048] cast-load replaces the 2048x64B descriptor bomb
            # and the on-chip B2T transpose build
            B2T = singles.tile([RANK, D], BF16)
            BTs = singles.tile([RANK, D], BF16)
            nc.gpsimd.dma_start(out=BTs[:], in_=BT_d[:])

            bb = singles.tile([P, D], BF16)
            ones = singles.tile([1, P], BF16)
            nc.vector.memset(ones[:], 1.0)
            b_sb = singles.tile([1, D], BF16)

            # both transposed operands fully resident in SBUF
            wT = wtp.tile([P, ND, D], BF16)  # wT[q, dt, o] = w[o, dt*128+q]
            xT = xtp.tile([P, ND, S], BF16)  # xT[q, dt, s] = x[s, dt*128+q]

            with (
                tc.tile_pool(name="wrow", bufs=3) as wrowp,
                tc.tile_pool(name="w16", bufs=2) as w16p,
                tc.tile_pool(name="xstage", bufs=5) as xstage,
                tc.tile_pool(name="yout", bufs=3) as youtp,
                tc.tile_pool(name="dpsum", bufs=3, space="PSUM") as dpsum,
                tc.tile_pool(name="tpsum", bufs=3, space="PSUM") as tpsum,
                tc.tile_pool(name="gpsum", bufs=2, space="PSUM") as gpsum,
            ):
                def x_load(st):
                    xs = xstage.tile([P, D], BF16, tag="xs")
                    nc.gpsimd.dma_start(
                        out=xs[:], in_=x_d[st * P : (st + 1) * P, :]
                    )
                    return xs

                # gpsimd queue order: A, BT, b (tiny), x0, x1, x2, ...
                # bias broadcast is a rank-1 PE matmul (ones x b) instead
                # of a 1MB SWDGE broadcast DMA: every x chunk lands ~4us
                # earlier and bb is ready by ~16us
                nc.gpsimd.dma_start(out=b_sb[:], in_=b_d[None, :])
                xs_tiles = {0: x_load(0)}
                xs_tiles[1] = x_load(1)
                xs_tiles[2] = x_load(2)

                # W row-blocks, evens on scalar / odds on sync; pool-gated
                # so later groups stream in during the GEMM passes
                wrows = []
                for ot in range(ND):
                    wrow = wrowp.tile([P, D], F32, tag="wrow")
                    eng = nc.scalar if ot % 2 == 0 else nc.sync
                    eng.dma_start(
                        out=wrow[:], in_=W_d[ot * P : (ot + 1) * P, :]
                    )
                    wrows.append(wrow)

                # 2*B.T via one DVE scale of the host-repacked B.T
                nc.vector.tensor_scalar_mul(B2T[:], BTs[:], SCALE)

                # bias broadcast: bb[p, o] = 1[p] * b[o]
                for g in range(NO):
                    bp = tpsum.tile([P, FREE], F32, tag="tp")
                    nc.tensor.matmul(
                        bp[:],
                        ones[:],
                        b_sb[:, g * FREE : (g + 1) * FREE],
                        start=True,
                        stop=True,
                    )
                    nc.scalar.copy(bb[:, g * FREE : (g + 1) * FREE], bp[:])

                def x_transpose(st):
                    xs = xs_tiles.pop(st) if st in xs_tiles else x_load(st)
                    for g in range(NG):
                        tp = tpsum.tile([P, 4 * P], F32, tag="tp")
                        for j in range(4):
                            dt = 4 * g + j
                            nc.tensor.matmul(
                                tp[:, j * P : (j + 1) * P],
                                xs[:, dt * P : (dt + 1) * P],
                                ident[:],
                                start=True,
                                stop=True,
                            )
                        nc.scalar.copy(
                            xT[:, 4 * g : 4 * (g + 1), st * P : (st + 1) * P],
                            tp[:],
                        )

                def w_compute(ot):
                    w16 = w16p.tile([P, D], BF16, tag="w16")
                    dps = [
                        dpsum.tile([P, FREE], F32, tag="dp", name=f"dp{ot}_{g}")
                        for g in range(NG)
                    ]
                    for g in range(NG):
                        nc.tensor.matmul(
                            dps[g][:],
                            B2T[:, ot * P : (ot + 1) * P],
                            A_sb[:, g * FREE : (g + 1) * FREE],
                            start=True,
                            stop=True,
                        )
                    for g in range(NG):
                        nc.vector.tensor_add(
                            w16[:, g * FREE : (g + 1) * FREE],
                            dps[g][:],
                            wrows[ot][:, g * FREE : (g + 1) * FREE],
                        )
                    for g in range(NG):
                        tp = tpsum.tile([P, 4 * P], F32, tag="tp")
                        for j in range(4):
                            dt = 4 * g + j
                            nc.tensor.matmul(
                                tp[:, j * P : (j + 1) * P],
                                w16[:, dt * P : (dt + 1) * P],
                                ident[:],
                                start=True,
                                stop=True,
                            )
                        nc.vector.tensor_scalar_mul(
                            wT[:, 4 * g : 4 * (g + 1), ot * P : (ot + 1) * P],
                            tp[:],
                            1.0,
                        )

                # group 0 prep + the first x tiles up front
                for ot in range(4):
                    w_compute(ot)
                    x_transpose(ot)

                # ---- oc-outer GEMM: pass oc needs W blocks 4oc..4oc+3;
                # group oc+1 prep and remaining x transposes are sprinkled
                # inside pass oc ----
                nxt_x = 4
                nxt_w = 4
                for oc in range(NO):
                    for st in range(NS):
                        if oc == 0 and st >= 2 and nxt_x < NS:
                            x_transpose(nxt_x)
                            nxt_x += 1
                        wsts = (8, 10, 12, 14) if oc == 0 else (4, 7, 10, 13)
                        if st in wsts and nxt_w < ND:
                            w_compute(nxt_w)
                            nxt_w += 1
                        gp = gpsum.tile([P, FREE], F32)
                        for dt in range(ND):
                            nc.tensor.matmul(
                                gp[:],
                                xT[:, dt, st * P : (st + 1) * P],
                                wT[:, dt, oc * FREE : (oc + 1) * FREE],
                                start=(dt == 0),
                                stop=(dt == ND - 1),
                            )
                        ys = youtp.tile([P, FREE], F32, tag="ys")
                        nc.vector.tensor_add(
                            ys[:], gp[:], bb[:, oc * FREE : (oc + 1) * FREE]
                        )
                        nc.sync.dma_start(
                            out=out_d[
                                st * P : (st + 1) * P,
                                oc * FREE : (oc + 1) * FREE,
                            ],
                            in_=ys[:],
                        )

    nc.compile()
    return nc


_NC_CACHE = None


def _get_nc():
    global _NC_CACHE
    if _NC_CACHE is None:
        _NC_CACHE = build_nc()
    return _NC_CACHE


def make_in_maps(x, W, b, B, A):
    x = np.ascontiguousarray(x, dtype=np.float32)
    W = np.ascontiguousarray(W, dtype=np.float32)
    b = np.ascontiguousarray(b, dtype=np.float32)
    B = np.ascontiguousarray(B, dtype=np.float32)
    A = np.ascontiguousarray(A, dtype=np.float32)
    BT = np.ascontiguousarray(B.T)  # layout repack only
    return [
        {"x": x[i], "W": W, "b": b, "BT": BT, "A": A} for i in range(N_CORES)
    ]


def run(inputs, **spmd_kwargs):
    """Run the SPMD kernel; returns (output, BassKernelResults)."""
    nc = _get_nc()
    in_maps = make_in_maps(**inputs)
    res = run_bass_kernel_spmd(nc, in_maps, core_ids=list(range(N_CORES)), **spmd_kwargs)
    out = np.stack([res.results[i]["out"] for i in range(N_CORES)]).astype(np.float32)
    return out, res


def kernel(x, W, b, B, A):
    out, _ = run({"x": x, "W": W, "b": b, "B": B, "A": A})
    return out
